# revision 4
# baseline (speedup 1.0000x reference)
"""Multi-head attention (B=2, S=2048, E=1024, H=16, D=64) on 8 Trainium2 NeuronCores.

Sharding (tensor-parallel over heads x data-parallel over batch, per the hint):
core c -> batch b=c//4, head-group g=c%4 (4 heads per core). Each core runs the
full per-group attention pipeline on device; the host sums the 4 partial
out-projections per batch element.

Device kernel (per core, transposed-scores formulation, fp32r matmuls):
  qkT[f, s]   = (W_qk_aug.T @ xT_aug)     f in [q(256)|k(256)], W_q prescaled by 1/sqrt(D)
  v[s, f]     = (xT_aug.T @ Wv_aug)       Wv augmented with unit columns so each
                                          head's [128,4,65] v-tile carries a ones col
  sT[k, q]    = kT_j slices @ qT_j        head pairs packed concurrently on PE row
                                          strips into one [128,1024] PSUM tile
  es          = exp(sT)                   one ACT op per [128,1024] tile (no max-sub;
                                          scores are ~N(0,1))
  poT[65, q]  = [v_j|1].T @ es-half       accumulated over k-tiles; row 64 = softmax denom
  aT          = poT[0:64] * recip(denom)  denom broadcast across partitions via DRAM bounce
  y[s, e]     = aT-chunks.T @ Wo_aug      partial out-projection (+b_out on group-0 cores)
Biases are folded in exactly via augmentation rows/columns (ones row appended to xT).
"""

import numpy as np
import concourse.bass as bass
import concourse.tile as tile
from concourse import bacc, mybir
from concourse.bass_utils import run_bass_kernel_spmd

F32 = mybir.dt.float32
F32R = mybir.dt.float32r

S = 2048
E = 1024
HL = 4        # heads per core
D = 64
QB = 512
NQB = S // QB
NKT = S // 128
NST = S // 128
NKC = E // 128

_CACHE = {}


def _build(repeat=1):
    nc = bacc.Bacc("TRN2", target_bir_lowering=False, debug=False, num_devices=8)

    xT_d = nc.dram_tensor("xT", [E, S], F32R, kind="ExternalInput").ap()
    wqk_d = nc.dram_tensor("wqk", [E + 1, 512], F32R, kind="ExternalInput").ap()
    wv_d = nc.dram_tensor("wv", [E + 1, HL * 65], F32R, kind="ExternalInput").ap()
    wo_d = nc.dram_tensor("wo", [257, E], F32R, kind="ExternalInput").ap()
    ones_d = nc.dram_tensor("ones", [1, S], F32R, kind="ExternalInput").ap()
    y_d = nc.dram_tensor("y", [S, E], F32, kind="ExternalOutput").ap()
    den_d = nc.dram_tensor("den_scratch", [HL * NQB, QB], F32)

    with tile.TileContext(nc) as tc:
        with (
            tc.tile_pool(name="wpool", bufs=1) as wpool,
            tc.tile_pool(name="big", bufs=1) as big,
            tc.tile_pool(name="xpool", bufs=12) as xpool,
            tc.tile_pool(name="espool", bufs=3) as espool,
            tc.tile_pool(name="denpool", bufs=4) as denpool,
            tc.tile_pool(name="bcpool", bufs=4) as bcpool,
            tc.tile_pool(name="ypool", bufs=3) as ypool,
            tc.tile_pool(name="pp", bufs=2, space="PSUM") as pp,
            tc.tile_pool(name="pq", bufs=4, space="PSUM") as pq,
        ):
            wqk_t = []
            for k in range(NKC):
                w = wpool.tile([128, 512], F32R, name=f"wqk{k}")
                nc.sync.dma_start(w[:], wqk_d[k * 128:(k + 1) * 128, :])
                wqk_t.append(w)
            wqk_b = wpool.tile([1, 512], F32R, name="wqk_b")
            nc.sync.dma_start(wqk_b[:], wqk_d[E:E + 1, :])
            wv_t = []
            for k in range(NKC):
                w = wpool.tile([128, HL * 65], F32R, name=f"wv{k}")
                nc.sync.dma_start(w[:], wv_d[k * 128:(k + 1) * 128, :])
                wv_t.append(w)
            wv_b = wpool.tile([1, HL * 65], F32R, name="wv_b")
            nc.sync.dma_start(wv_b[:], wv_d[E:E + 1, :])
            wo_t = []
            for c in range(2):
                w = wpool.tile([128, E], F32R, name=f"wo{c}")
                nc.sync.dma_start(w[:], wo_d[c * 128:(c + 1) * 128, :])
                wo_t.append(w)
            wo_b = wpool.tile([1, E], F32R, name="wo_b")
            nc.sync.dma_start(wo_b[:], wo_d[256:257, :])
            ones_r = wpool.tile([1, S], F32R, name="ones_r")
            nc.sync.dma_start(ones_r[:], ones_d[:])

            qkT = [big.tile([128, S], F32R, name=f"qkT{m}") for m in range(4)]
            vt = [big.tile([128, HL, 65], F32R, name=f"vt{st}") for st in range(NST)]
            a_t = [big.tile([128, S], F32R, name=f"a{c}") for c in range(2)]

            def _iter_body():
                # ===== P1: projections =====
                for qb in range(NQB):
                    xc = []
                    for k in range(NKC):
                        x = xpool.tile([128, QB], F32R, name="xc")
                        nc.sync.dma_start(x[:], xT_d[k * 128:(k + 1) * 128, qb * QB:(qb + 1) * QB])
                        xc.append(x)
                    for m in range(4):
                        p = pp.tile([128, QB], F32, name="pp_t", tag="pp")
                        for k in range(NKC):
                            nc.tensor.matmul(p[:], wqk_t[k][:, m * 128:(m + 1) * 128],
                                             xc[k][:], start=(k == 0), stop=False)
                        nc.tensor.matmul(p[:], wqk_b[:, m * 128:(m + 1) * 128],
                                         ones_r[:, qb * QB:(qb + 1) * QB],
                                         start=False, stop=True)
                        nc.vector.tensor_copy(qkT[m][:, qb * QB:(qb + 1) * QB], p[:])
                    for sl in range(4):
                        st = qb * 4 + sl
                        p = pq.tile([128, HL * 65], F32, name="pq_t", tag="qq")
                        for k in range(NKC):
                            nc.tensor.matmul(p[:], xc[k][:, sl * 128:(sl + 1) * 128],
                                             wv_t[k][:], start=(k == 0), stop=False)
                        nc.tensor.matmul(p[:], ones_r[:, st * 128:(st + 1) * 128], wv_b[:],
                                         start=False, stop=True)
                        nc.vector.tensor_copy(vt[st][:, :, :].rearrange("p h d -> p (h d)"), p[:])

                # ===== P2: attention, head pairs packed on PE row strips =====
                for pr in range(2):
                    j0, j1 = 2 * pr, 2 * pr + 1
                    qT0 = qkT[pr][0:64, :]
                    qT1 = qkT[pr][64:128, :]
                    kT0 = qkT[2 + pr][0:64, :]
                    kT1 = qkT[2 + pr][64:128, :]
                    for qb in range(NQB):
                        po_a = pq.tile([65, QB], F32, name="po_a", tag="qq")
                        po_b = pq.tile([65, QB], F32, name="po_b", tag="qq")
                        for kt in range(NKT):
                            ps_t = pp.tile([128, 2 * QB], F32, name="ps_t", tag="pp")
                            nc.tensor.matmul(ps_t[:, 0:QB], kT0[:, kt * 128:(kt + 1) * 128],
                                             qT0[:, qb * QB:(qb + 1) * QB], start=True, stop=True)
                            nc.tensor.matmul(ps_t[:, QB:2 * QB], kT1[:, kt * 128:(kt + 1) * 128],
                                             qT1[:, qb * QB:(qb + 1) * QB], start=True, stop=True)
                            es = espool.tile([128, 2 * QB], F32R, name="es")
                            nc.scalar.activation(es[:], ps_t[:], mybir.ActivationFunctionType.Exp)
                            nc.tensor.matmul(po_a[:], vt[kt][:, j0, :], es[:, 0:QB],
                                             start=(kt == 0), stop=(kt == NKT - 1))
                            nc.tensor.matmul(po_b[:], vt[kt][:, j1, :], es[:, QB:2 * QB],
                                             start=(kt == 0), stop=(kt == NKT - 1))
                        for j, po_t in ((j0, po_a), (j1, po_b)):
                            den_r = denpool.tile([1, QB], F32, name="den_r")
                            nc.vector.reciprocal(den_r[:], po_t[64:65, :])
                            slot = j * NQB + qb
                            nc.sync.dma_start(den_d[slot:slot + 1, :], den_r[:])
                            bc = bcpool.tile([64, QB], F32, name="bc")
                            nc.sync.dma_start(bc[:], den_d[slot:slot + 1, :].to_broadcast((64, QB)))
                            a_out = a_t[j // 2][(j % 2) * 64:(j % 2) * 64 + 64,
                                                qb * QB:(qb + 1) * QB]
                            nc.vector.tensor_mul(a_out, po_t[0:64, :], bc[:])

                # ===== P3: out-projection =====
                for st in range(NST):
                    for n in range(2):
                        p = pp.tile([128, QB], F32, name="pp_t", tag="pp")
                        nc.tensor.matmul(p[:], a_t[0][:, st * 128:(st + 1) * 128],
                                         wo_t[0][:, n * QB:(n + 1) * QB], start=True, stop=False)
                        nc.tensor.matmul(p[:], a_t[1][:, st * 128:(st + 1) * 128],
                                         wo_t[1][:, n * QB:(n + 1) * QB], start=False, stop=False)
                        nc.tensor.matmul(p[:], ones_r[:, st * 128:(st + 1) * 128],
                                         wo_b[:, n * QB:(n + 1) * QB], start=False, stop=True)
                        yt = ypool.tile([128, QB], F32, name="yt")
                        nc.vector.tensor_copy(yt[:], p[:])
                        nc.sync.dma_start(y_d[st * 128:(st + 1) * 128, n * QB:(n + 1) * QB], yt[:])

            if repeat == 1:
                _iter_body()
            else:
                with tc.For_i(0, repeat):
                    _iter_body()

    nc.compile()
    return nc


def _shard_inputs(query, W_qkv, b_qkv, W_out, b_out):
    scale = np.float32(1.0 / np.sqrt(D))
    query = np.asarray(query, dtype=np.float32)
    W_qkv = np.asarray(W_qkv, dtype=np.float32)
    b_qkv = np.asarray(b_qkv, dtype=np.float32)
    W_out = np.asarray(W_out, dtype=np.float32)
    b_out = np.asarray(b_out, dtype=np.float32)

    W_q, W_k, W_v = W_qkv[:, :E], W_qkv[:, E:2 * E], W_qkv[:, 2 * E:]
    b_q, b_k, b_v = b_qkv[:E], b_qkv[E:2 * E], b_qkv[2 * E:]

    ones = np.ones((1, S), np.float32)
    in_maps = []
    for c in range(8):
        b = c // 4
        g = c % 4
        hsl = slice(4 * g * D, (4 * g + 4) * D)
        wqk = np.empty((E + 1, 512), np.float32)
        wqk[:E, :256] = W_q[:, hsl] * scale
        wqk[E, :256] = b_q[hsl] * scale
        wqk[:E, 256:] = W_k[:, hsl]
        wqk[E, 256:] = b_k[hsl]
        wv = np.zeros((E + 1, HL * 65), np.float32)
        for j in range(HL):
            js = slice(4 * g * D + j * D, 4 * g * D + (j + 1) * D)
            wv[:E, j * 65:j * 65 + 64] = W_v[:, js]
            wv[E, j * 65:j * 65 + 64] = b_v[js]
            wv[E, j * 65 + 64] = 1.0
        wo = np.empty((257, E), np.float32)
        wo[:256] = W_out[hsl, :]
        wo[256] = b_out if g == 0 else 0.0
        in_maps.append({
            "xT": np.ascontiguousarray(query[b].T),
            "wqk": wqk,
            "wv": wv,
            "wo": wo,
            "ones": ones,
        })
    return in_maps


def kernel(query, W_qkv, b_qkv, W_out, b_out):
    if "nc" not in _CACHE:
        _CACHE["nc"] = _build()
    nc = _CACHE["nc"]
    in_maps = _shard_inputs(query, W_qkv, b_qkv, W_out, b_out)
    res = run_bass_kernel_spmd(nc, in_maps, list(range(8)))
    out = np.zeros((2, S, E), np.float32)
    for c in range(8):
        out[c // 4] += res.results[c]["y"]
    return out


# revision 13
# speedup vs baseline: 1.0061x; 1.0061x over previous
"""Multi-head attention (B=2, S=2048, E=1024, H=16, D=64) on 8 Trainium2 NeuronCores.

Sharding: core c -> batch b=c//4, head-group g=c%4 (4 heads per core). Each core
runs the full per-group attention pipeline; the host sums the 4 partial
out-projections per batch element.

v2 kernel (bf16 matmuls, software-pipelined phases):
  - All matmul operands bf16 (weights + x converted on host); PSUM accumulation f32.
  - Projections produce qT/kT per head-pair ([128,512] tiles per q-block) and
    v per seq-chunk ([128,4,65] tiles, 65th col = exact ones for the softmax denom).
  - Biases are folded into the PSUM->SBUF epilogues (DVE tensor_scalar_add with a
    per-partition bias column for qT/kT; tensor_add with broadcast tiles for v/y),
    costing zero PE cycles while staying exact.
  - Attention per (pair, q-block): sT = kT_j @ qT_j per 128-key tile -> exp on ACT
    (bf16 out) -> po[65,512] accumulated with v stationary; row 64 = denominator.
  - Normalization: DVE reciprocal -> PE rank-1 broadcast (ones ⊗ recip) -> DVE mul
    writes aT slices (no DRAM round-trip).
  - Issue order software-pipelines the phases: pair-0 projections + v first, the
    pair-1 projections interleave into pair-0's attention loop, and each q-block's
    out-projection interleaves behind its attention, so PE stays busy during the
    ACT-heavy stretches. A tiny exp at t=0 preloads the ACT table during DMA waits.
"""

import numpy as np
import concourse.bass as bass
import concourse.tile as tile
from concourse import bacc, mybir
from concourse.bass_utils import run_bass_kernel_spmd

F32 = mybir.dt.float32
F32R = mybir.dt.float32r
BF16 = mybir.dt.bfloat16
NPBF16 = mybir.dt.np(mybir.dt.bfloat16)

S = 2048
E = 1024
HL = 4        # heads per core
D = 64
QB = 512
NQB = S // QB      # 4
NKT = S // 128     # 16
NKC = E // 128     # 8

_CACHE = {}


def _build(repeat=1):
    nc = bacc.Bacc("TRN2", target_bir_lowering=False, debug=False, num_devices=8)

    xT_d = nc.dram_tensor("xT", [E, S], BF16, kind="ExternalInput").ap()
    wqk_d = nc.dram_tensor("wqk", [E, 512], BF16, kind="ExternalInput").ap()
    bqk_d = nc.dram_tensor("bqk", [128, 4], F32, kind="ExternalInput").ap()
    wv_d = nc.dram_tensor("wv", [E, 256], BF16, kind="ExternalInput").ap()
    bv_d = nc.dram_tensor("bv", [1, 256], F32, kind="ExternalInput").ap()
    wo_d = nc.dram_tensor("wo", [256, E], BF16, kind="ExternalInput").ap()
    bo_d = nc.dram_tensor("bo", [1, E], F32, kind="ExternalInput").ap()
    y_d = nc.dram_tensor("y", [S, E], F32, kind="ExternalOutput").ap()

    with tile.TileContext(nc) as tc:
        with (
            nc.allow_low_precision(reason="bf16 activations by design; f32 accum in PSUM"),
            tc.tile_pool(name="wpool", bufs=1) as wpool,
            tc.tile_pool(name="big", bufs=1) as big,
            tc.tile_pool(name="espool", bufs=6) as espool,
            tc.tile_pool(name="denpool", bufs=4) as denpool,
            tc.tile_pool(name="ypool", bufs=3) as ypool,
            tc.tile_pool(name="ps", bufs=3, space="PSUM") as ps,
            tc.tile_pool(name="pq", bufs=3, space="PSUM") as pq,
            tc.tile_pool(name="pr", bufs=2, space="PSUM") as pr,
        ):
            # ---- persistent weights (outside the repeat loop) ----
            wqk_t = []
            for k in range(NKC):
                w = wpool.tile([128, 512], BF16, name=f"wqk{k}")
                nc.sync.dma_start(w[:], wqk_d[k * 128:(k + 1) * 128, :])
                wqk_t.append(w)
            bqk_t = wpool.tile([128, 4], F32, name="bqk_t")
            nc.sync.dma_start(bqk_t[:], bqk_d[:])
            wv_t = []
            for k in range(NKC):
                w = wpool.tile([128, 256], BF16, name=f"wv{k}")
                nc.sync.dma_start(w[:], wv_d[k * 128:(k + 1) * 128, :])
                wv_t.append(w)
            bv_bc = wpool.tile([128, 256], F32, name="bv_bc")
            nc.sync.dma_start(bv_bc[:], bv_d[0:1, :].to_broadcast((128, 256)))
            wo_t = []
            for c in range(2):
                w = wpool.tile([128, E], BF16, name=f"wo{c}")
                nc.sync.dma_start(w[:], wo_d[c * 128:(c + 1) * 128, :])
                wo_t.append(w)
            bo_bc = wpool.tile([128, E], F32, name="bo_bc")
            nc.sync.dma_start(bo_bc[:], bo_d[0:1, :].to_broadcast((128, E)))

            # ACT exp-table warmup: runs during the weight DMAs
            warm = wpool.tile([1, 8], F32, name="warm")
            nc.vector.memset(warm[:], 0.0)
            nc.scalar.activation(warm[:], warm[:], mybir.ActivationFunctionType.Exp)

            # ---- persistent per-iteration tiles ----
            # x_t[k][qb]: xT chunk [128, 512]
            x_t = [[big.tile([128, QB], BF16, name=f"x{k}_{qb}") for qb in range(NQB)]
                   for k in range(NKC)]
            # qk_t[m][qb]: m=0 qT pair0, 1 qT pair1, 2 kT pair0, 3 kT pair1
            qk_t = [[big.tile([128, QB], BF16, name=f"qk{m}_{qb}") for qb in range(NQB)]
                    for m in range(4)]
            vt = [big.tile([128, HL, 65], BF16, name=f"vt{st}") for st in range(NKT)]
            a_t = [big.tile([128, S], BF16, name=f"a{c}") for c in range(2)]

            def load_x():
                for qb in range(NQB):
                    for k in range(NKC):
                        nc.sync.dma_start(
                            x_t[k][qb][:],
                            xT_d[k * 128:(k + 1) * 128, qb * QB:(qb + 1) * QB])

            def qk_proj(m, qb):
                p = pr.tile([128, QB], F32, name="prt", tag="r")
                for k in range(NKC):
                    nc.tensor.matmul(p[:], wqk_t[k][:, m * 128:(m + 1) * 128],
                                     x_t[k][qb][:], start=(k == 0), stop=(k == NKC - 1))
                nc.vector.tensor_scalar_add(qk_t[m][qb][:], p[:], bqk_t[:, m:m + 1])

            def v_proj(st):
                qb, sl = st // 4, st % 4
                p = pr.tile([128, QB], F32, name="prt", tag="r")
                pv = p[:, 0:256]
                for k in range(NKC):
                    nc.tensor.matmul(pv, x_t[k][qb][:, sl * 128:(sl + 1) * 128],
                                     wv_t[k][:], start=(k == 0), stop=(k == NKC - 1))
                nc.vector.tensor_add(
                    vt[st][:, :, 0:64],
                    pv.rearrange("p (h d) -> p h d", h=HL),
                    bv_bc[:].rearrange("p (h d) -> p h d", h=HL))
                nc.vector.memset(vt[st][:, :, 64:65], 1.0)

            def attn_block(pr_i, qb, interleave):
                """Scores+exp+po for head pair pr_i on q-block qb; `interleave` is a
                list of (after_kt, fn) callbacks issued inside the kt loop."""
                j0, j1 = 2 * pr_i, 2 * pr_i + 1
                po_a = pq.tile([65, QB], F32, name="po_a", tag="q")
                po_b = pq.tile([65, QB], F32, name="po_b", tag="q")
                cbs = dict(interleave)
                for kt in range(NKT):
                    qt_c, cc = kt // 4, (kt % 4) * 128
                    ps_a = ps.tile([128, QB], F32, name="ps_a", tag="s")
                    nc.tensor.matmul(ps_a[:], qk_t[2 + pr_i][qt_c][0:64, cc:cc + 128],
                                     qk_t[pr_i][qb][0:64, :], start=True, stop=True)
                    es_a = espool.tile([128, QB], BF16, name="es_a")
                    nc.scalar.activation(es_a[:], ps_a[:], mybir.ActivationFunctionType.Exp)
                    nc.tensor.matmul(po_a[:], vt[kt][:, j0, :], es_a[:],
                                     start=(kt == 0), stop=(kt == NKT - 1))
                    ps_b = ps.tile([128, QB], F32, name="ps_b", tag="s")
                    nc.tensor.matmul(ps_b[:], qk_t[2 + pr_i][qt_c][64:128, cc:cc + 128],
                                     qk_t[pr_i][qb][64:128, :], start=True, stop=True)
                    es_b = espool.tile([128, QB], BF16, name="es_b")
                    nc.scalar.activation(es_b[:], ps_b[:], mybir.ActivationFunctionType.Exp)
                    nc.tensor.matmul(po_b[:], vt[kt][:, j1, :], es_b[:],
                                     start=(kt == 0), stop=(kt == NKT - 1))
                    if kt in cbs:
                        cbs[kt]()
                for half, po_t in ((0, po_a), (1, po_b)):
                    den_r = denpool.tile([1, QB], F32, name="den_r")
                    nc.vector.reciprocal(den_r[:], po_t[64:65, :])
                    den_sb = denpool.tile([64, QB], F32, name="den_sb")
                    nc.gpsimd.partition_broadcast(den_sb[:], den_r[:])
                    a_out = a_t[pr_i][half * 64:half * 64 + 64, qb * QB:(qb + 1) * QB]
                    nc.vector.tensor_mul(a_out, po_t[0:64, :], den_sb[:])

            def out_proj_sl(st):
                for n in range(2):
                    p = pr.tile([128, QB], F32, name="prt", tag="r")
                    nc.tensor.matmul(p[:], a_t[0][:, st * 128:(st + 1) * 128],
                                     wo_t[0][:, n * QB:(n + 1) * QB],
                                     start=True, stop=False)
                    nc.tensor.matmul(p[:], a_t[1][:, st * 128:(st + 1) * 128],
                                     wo_t[1][:, n * QB:(n + 1) * QB],
                                     start=False, stop=True)
                    yt = ypool.tile([128, QB], F32, name="yt")
                    nc.vector.tensor_add(yt[:], p[:], bo_bc[:, n * QB:(n + 1) * QB])
                    nc.sync.dma_start(
                        y_d[st * 128:(st + 1) * 128, n * QB:(n + 1) * QB], yt[:])

            def _iter_body():
                load_x()
                # stage A: pair-0 q/k projections + all of v
                for qb in range(NQB):
                    qk_proj(0, qb)
                    qk_proj(2, qb)
                    for sl in range(4):
                        v_proj(qb * 4 + sl)
                # pair-0 attention, pair-1 projections interleaved inside
                for qb in range(NQB):
                    il = [(5, lambda qb=qb: qk_proj(1, qb)),
                          (11, lambda qb=qb: qk_proj(3, qb))]
                    attn_block(0, qb, il)
                # pair-1 attention, out-projections of the previous q-block inside
                for qb in range(NQB):
                    il = [(2 + 4 * sl, lambda st=(qb - 1) * 4 + sl: out_proj_sl(st))
                          for sl in range(4)] if qb > 0 else []
                    attn_block(1, qb, il)
                for sl in range(4):
                    out_proj_sl(12 + sl)

            if repeat == 1:
                _iter_body()
            else:
                with tc.For_i(0, repeat):
                    _iter_body()

    nc.compile()
    return nc


def _shard_inputs(query, W_qkv, b_qkv, W_out, b_out):
    scale = np.float32(1.0 / np.sqrt(D))
    query = np.asarray(query, dtype=np.float32)
    W_qkv = np.asarray(W_qkv, dtype=np.float32)
    b_qkv = np.asarray(b_qkv, dtype=np.float32)
    W_out = np.asarray(W_out, dtype=np.float32)
    b_out = np.asarray(b_out, dtype=np.float32)

    W_q, W_k, W_v = W_qkv[:, :E], W_qkv[:, E:2 * E], W_qkv[:, 2 * E:]
    b_q, b_k, b_v = b_qkv[:E], b_qkv[E:2 * E], b_qkv[2 * E:]

    in_maps = []
    for c in range(8):
        b = c // 4
        g = c % 4
        hsl = slice(4 * g * D, (4 * g + 4) * D)
        wqk = np.empty((E, 512), np.float32)
        wqk[:, :256] = W_q[:, hsl] * scale
        wqk[:, 256:] = W_k[:, hsl]
        bqk_cols = np.empty((512,), np.float32)
        bqk_cols[:256] = b_q[hsl] * scale
        bqk_cols[256:] = b_k[hsl]
        bqk = np.ascontiguousarray(bqk_cols.reshape(4, 128).T)
        in_maps.append({
            "xT": np.ascontiguousarray(query[b].T).astype(NPBF16),
            "wqk": wqk.astype(NPBF16),
            "bqk": bqk,
            "wv": np.ascontiguousarray(W_v[:, hsl]).astype(NPBF16),
            "bv": np.ascontiguousarray(b_v[hsl]).reshape(1, 256),
            "wo": np.ascontiguousarray(W_out[hsl, :]).astype(NPBF16),
            "bo": (b_out if g == 0 else np.zeros_like(b_out)).reshape(1, E),
        })
    return in_maps


def kernel(query, W_qkv, b_qkv, W_out, b_out):
    if "nc" not in _CACHE:
        _CACHE["nc"] = _build()
    nc = _CACHE["nc"]
    in_maps = _shard_inputs(query, W_qkv, b_qkv, W_out, b_out)
    res = run_bass_kernel_spmd(nc, in_maps, list(range(8)))
    out = np.zeros((2, S, E), np.float32)
    for c in range(8):
        out[c // 4] += res.results[c]["y"]
    return out


# revision 20
# speedup vs baseline: 1.2043x; 1.1970x over previous
"""Multi-head attention (B=2, S=2048, E=1024, H=16, D=64) on 8 Trainium2 NeuronCores.

Sharding: core c -> batch b=c//4, head-group g=c%4 (4 heads per core). Each core
runs the full per-group attention pipeline; the host sums the 4 partial
out-projections per batch element.

v2 kernel (bf16 matmuls, software-pipelined phases):
  - All matmul operands bf16 (weights + x converted on host); PSUM accumulation f32.
  - Projections produce qT/kT per head-pair ([128,512] tiles per q-block) and
    v per seq-chunk ([128,4,65] tiles, 65th col = exact ones for the softmax denom).
  - Biases are folded into the PSUM->SBUF epilogues (DVE tensor_scalar_add with a
    per-partition bias column for qT/kT; tensor_add with broadcast tiles for v/y),
    costing zero PE cycles while staying exact.
  - Attention per (pair, q-block): sT = kT_j @ qT_j per 128-key tile -> exp on ACT
    (bf16 out) -> po[65,512] accumulated with v stationary; row 64 = denominator.
  - Normalization: DVE reciprocal -> PE rank-1 broadcast (ones ⊗ recip) -> DVE mul
    writes aT slices (no DRAM round-trip).
  - Issue order software-pipelines the phases: pair-0 projections + v first, the
    pair-1 projections interleave into pair-0's attention loop, and each q-block's
    out-projection interleaves behind its attention, so PE stays busy during the
    ACT-heavy stretches. A tiny exp at t=0 preloads the ACT table during DMA waits.
"""

import numpy as np
import concourse.bass as bass
import concourse.tile as tile
from concourse import bacc, mybir
from concourse.bass_utils import run_bass_kernel_spmd

F32 = mybir.dt.float32
F32R = mybir.dt.float32r
BF16 = mybir.dt.bfloat16
NPBF16 = mybir.dt.np(mybir.dt.bfloat16)

S = 2048
E = 1024
HL = 4        # heads per core
D = 64
QB = 512
NQB = S // QB      # 4
NKT = S // 128     # 16
NKC = E // 128     # 8

_CACHE = {}


def _build(repeat=1):
    nc = bacc.Bacc("TRN2", target_bir_lowering=False, debug=False, num_devices=8)

    xT_d = nc.dram_tensor("xT", [E, S], BF16, kind="ExternalInput").ap()
    wqk_d = nc.dram_tensor("wqk", [E, 512], BF16, kind="ExternalInput").ap()
    bqk_d = nc.dram_tensor("bqk", [128, 4], F32, kind="ExternalInput").ap()
    wv_d = nc.dram_tensor("wv", [E, 256], BF16, kind="ExternalInput").ap()
    bv_d = nc.dram_tensor("bv", [1, 256], F32, kind="ExternalInput").ap()
    wo_d = nc.dram_tensor("wo", [256, E], BF16, kind="ExternalInput").ap()
    bo_d = nc.dram_tensor("bo", [1, E], F32, kind="ExternalInput").ap()
    onesc_d = nc.dram_tensor("onesc", [1, 64], F32R, kind="ExternalInput").ap()
    y_d = nc.dram_tensor("y", [S, E], F32, kind="ExternalOutput").ap()

    with tile.TileContext(nc) as tc:
        with (
            nc.allow_low_precision(reason="bf16 activations by design; f32 accum in PSUM"),
            tc.tile_pool(name="wpool", bufs=1) as wpool,
            tc.tile_pool(name="big", bufs=1) as big,
            tc.tile_pool(name="espool", bufs=5) as espool,
            tc.tile_pool(name="denpool", bufs=4) as denpool,
            tc.tile_pool(name="araw", bufs=4) as araw,
            tc.tile_pool(name="ypool", bufs=3) as ypool,
            tc.tile_pool(name="ps", bufs=2, space="PSUM") as ps,
            tc.tile_pool(name="pq", bufs=2, space="PSUM") as pq,
            tc.tile_pool(name="pr", bufs=2, space="PSUM") as pr,
        ):
            # ---- persistent weights (outside the repeat loop) ----
            # order: everything stage A needs first (wqk, bqk, wv, bv), then the rest
            wqk_t = []
            for k in range(NKC):
                w = wpool.tile([128, 512], BF16, name=f"wqk{k}")
                nc.sync.dma_start(w[:], wqk_d[k * 128:(k + 1) * 128, :])
                wqk_t.append(w)
            bqk_t = wpool.tile([128, 4], F32, name="bqk_t")
            nc.sync.dma_start(bqk_t[:], bqk_d[:])
            wv_t = []
            for k in range(NKC):
                w = wpool.tile([128, 256], BF16, name=f"wv{k}")
                nc.sync.dma_start(w[:], wv_d[k * 128:(k + 1) * 128, :])
                wv_t.append(w)
            bv_bc = wpool.tile([128, 256], F32, name="bv_bc")
            nc.sync.dma_start(bv_bc[:], bv_d[0:1, :].to_broadcast((128, 256)))
            onesc_t = wpool.tile([1, 64], F32R, name="onesc_t")
            nc.sync.dma_start(onesc_t[:], onesc_d[:])
            wo_t = []
            for c in range(2):
                w = wpool.tile([128, E], BF16, name=f"wo{c}")
                nc.sync.dma_start(w[:], wo_d[c * 128:(c + 1) * 128, :])
                wo_t.append(w)
            bo_bc = wpool.tile([128, E], F32, name="bo_bc")
            nc.sync.dma_start(bo_bc[:], bo_d[0:1, :].to_broadcast((128, E)))

            # ACT exp-table warmup: runs during the weight DMAs
            warm = wpool.tile([1, 8], F32, name="warm")
            nc.vector.memset(warm[:], 0.0)
            nc.scalar.activation(warm[:], warm[:], mybir.ActivationFunctionType.Exp)

            # ---- persistent per-iteration tiles ----
            # x_t[k][qb]: xT chunk [128, 512]
            x_t = [[big.tile([128, QB], BF16, name=f"x{k}_{qb}") for qb in range(NQB)]
                   for k in range(NKC)]
            # qk_t[m][qb]: m=0 qT pair0, 1 qT pair1, 2 kT pair0, 3 kT pair1
            qk_t = [[big.tile([128, QB], BF16, name=f"qk{m}_{qb}") for qb in range(NQB)]
                    for m in range(4)]
            vt = [big.tile([128, HL, 65], BF16, name=f"vt{st}") for st in range(NKT)]
            a_t = [big.tile([128, S], BF16, name=f"a{c}") for c in range(2)]

            def load_x():
                for qb in range(NQB):
                    for k in range(NKC):
                        nc.gpsimd.dma_start(
                            x_t[k][qb][:],
                            xT_d[k * 128:(k + 1) * 128, qb * QB:(qb + 1) * QB])

            def qk_proj(m, qb):
                p = pr.tile([128, QB], F32, name="prt", tag="r")
                for k in range(NKC):
                    nc.tensor.matmul(p[:], wqk_t[k][:, m * 128:(m + 1) * 128],
                                     x_t[k][qb][:], start=(k == 0), stop=(k == NKC - 1))
                nc.vector.tensor_scalar_add(qk_t[m][qb][:], p[:], bqk_t[:, m:m + 1])

            def v_proj(st):
                qb, sl = st // 4, st % 4
                p = pr.tile([128, QB], F32, name="prt", tag="r")
                pv = p[:, 0:256]
                for k in range(NKC):
                    nc.tensor.matmul(pv, x_t[k][qb][:, sl * 128:(sl + 1) * 128],
                                     wv_t[k][:], start=(k == 0), stop=(k == NKC - 1))
                nc.vector.tensor_add(
                    vt[st][:, :, 0:64],
                    pv.rearrange("p (h d) -> p h d", h=HL),
                    bv_bc[:].rearrange("p (h d) -> p h d", h=HL))
                nc.vector.memset(vt[st][:, :, 64:65], 1.0)

            def attn_block(pr_i, qb, interleave):
                """Scores+exp+po for head pair pr_i on q-block qb; `interleave` is a
                list of (after_kt, fn) callbacks issued inside the kt loop."""
                j0, j1 = 2 * pr_i, 2 * pr_i + 1
                po_a = pq.tile([65, QB], F32, name="po_a", tag="q")
                po_b = pq.tile([65, QB], F32, name="po_b", tag="q")
                cbs = dict(interleave)
                for kt in range(NKT):
                    qt_c, cc = kt // 4, (kt % 4) * 128
                    ps_t = ps.tile([128, 2 * QB], F32, name="ps_t", tag="s")
                    nc.tensor.matmul(ps_t[:, 0:QB], qk_t[2 + pr_i][qt_c][0:64, cc:cc + 128],
                                     qk_t[pr_i][qb][0:64, :], start=True, stop=True)
                    nc.tensor.matmul(ps_t[:, QB:2 * QB],
                                     qk_t[2 + pr_i][qt_c][64:128, cc:cc + 128],
                                     qk_t[pr_i][qb][64:128, :], start=True, stop=True)
                    es = espool.tile([128, 2 * QB], BF16, name="es")
                    nc.scalar.activation(es[:], ps_t[:], mybir.ActivationFunctionType.Exp)
                    nc.tensor.matmul(po_a[:], vt[kt][:, j0, :], es[:, 0:QB],
                                     start=(kt == 0), stop=(kt == NKT - 1))
                    nc.tensor.matmul(po_b[:], vt[kt][:, j1, :], es[:, QB:2 * QB],
                                     start=(kt == 0), stop=(kt == NKT - 1))
                    if kt in cbs:
                        cbs[kt]()
                for half, po_t in ((0, po_a), (1, po_b)):
                    den_r = denpool.tile([1, QB], F32R, name="den_r")
                    nc.vector.reciprocal(den_r[:], po_t[64:65, :])
                    a_raw = araw.tile([64, QB], BF16, name="a_raw")
                    nc.vector.tensor_copy(a_raw[:], po_t[0:64, :])
                    den_bc = pr.tile([64, QB], F32, name="den_bc", tag="r")
                    nc.tensor.matmul(den_bc[:], onesc_t[:], den_r[:], start=True, stop=True)
                    a_out = a_t[pr_i][half * 64:half * 64 + 64, qb * QB:(qb + 1) * QB]
                    nc.vector.tensor_mul(a_out, a_raw[:], den_bc[:])

            def out_proj_sl(st):
                for n in range(2):
                    p = pr.tile([128, QB], F32, name="prt", tag="r")
                    nc.tensor.matmul(p[:], a_t[0][:, st * 128:(st + 1) * 128],
                                     wo_t[0][:, n * QB:(n + 1) * QB],
                                     start=True, stop=False)
                    nc.tensor.matmul(p[:], a_t[1][:, st * 128:(st + 1) * 128],
                                     wo_t[1][:, n * QB:(n + 1) * QB],
                                     start=False, stop=True)
                    yt = ypool.tile([128, QB], F32, name="yt")
                    nc.vector.tensor_add(yt[:], p[:], bo_bc[:, n * QB:(n + 1) * QB])
                    nc.gpsimd.dma_start(
                        y_d[st * 128:(st + 1) * 128, n * QB:(n + 1) * QB], yt[:])

            def _iter_body():
                load_x()
                # stage A: pair-0 q/k projections + all of v
                for qb in range(NQB):
                    qk_proj(0, qb)
                    qk_proj(2, qb)
                    for sl in range(4):
                        v_proj(qb * 4 + sl)
                # pair-0 attention, pair-1 projections interleaved inside
                for qb in range(NQB):
                    il = [(5, lambda qb=qb: qk_proj(1, qb)),
                          (11, lambda qb=qb: qk_proj(3, qb))]
                    attn_block(0, qb, il)
                # pair-1 attention, out-projections of the previous q-block inside
                for qb in range(NQB):
                    il = [(2 + 4 * sl, lambda st=(qb - 1) * 4 + sl: out_proj_sl(st))
                          for sl in range(4)] if qb > 0 else []
                    attn_block(1, qb, il)
                for sl in range(4):
                    out_proj_sl(12 + sl)

            if repeat == 1:
                _iter_body()
            else:
                with tc.For_i(0, repeat):
                    _iter_body()

    nc.compile()
    return nc


def _shard_inputs(query, W_qkv, b_qkv, W_out, b_out):
    scale = np.float32(1.0 / np.sqrt(D))
    query = np.asarray(query, dtype=np.float32)
    W_qkv = np.asarray(W_qkv, dtype=np.float32)
    b_qkv = np.asarray(b_qkv, dtype=np.float32)
    W_out = np.asarray(W_out, dtype=np.float32)
    b_out = np.asarray(b_out, dtype=np.float32)

    W_q, W_k, W_v = W_qkv[:, :E], W_qkv[:, E:2 * E], W_qkv[:, 2 * E:]
    b_q, b_k, b_v = b_qkv[:E], b_qkv[E:2 * E], b_qkv[2 * E:]

    in_maps = []
    for c in range(8):
        b = c // 4
        g = c % 4
        hsl = slice(4 * g * D, (4 * g + 4) * D)
        wqk = np.empty((E, 512), np.float32)
        wqk[:, :256] = W_q[:, hsl] * scale
        wqk[:, 256:] = W_k[:, hsl]
        bqk_cols = np.empty((512,), np.float32)
        bqk_cols[:256] = b_q[hsl] * scale
        bqk_cols[256:] = b_k[hsl]
        bqk = np.ascontiguousarray(bqk_cols.reshape(4, 128).T)
        in_maps.append({
            "xT": np.ascontiguousarray(query[b].T).astype(NPBF16),
            "wqk": wqk.astype(NPBF16),
            "bqk": bqk,
            "wv": np.ascontiguousarray(W_v[:, hsl]).astype(NPBF16),
            "bv": np.ascontiguousarray(b_v[hsl]).reshape(1, 256),
            "wo": np.ascontiguousarray(W_out[hsl, :]).astype(NPBF16),
            "bo": (b_out if g == 0 else np.zeros_like(b_out)).reshape(1, E),
            "onesc": np.ones((1, 64), np.float32),
        })
    return in_maps


def kernel(query, W_qkv, b_qkv, W_out, b_out):
    if "nc" not in _CACHE:
        _CACHE["nc"] = _build()
    nc = _CACHE["nc"]
    in_maps = _shard_inputs(query, W_qkv, b_qkv, W_out, b_out)
    res = run_bass_kernel_spmd(nc, in_maps, list(range(8)))
    out = np.zeros((2, S, E), np.float32)
    for c in range(8):
        out[c // 4] += res.results[c]["y"]
    return out


# revision 29
# speedup vs baseline: 1.2533x; 1.0406x over previous
"""Multi-head attention (B=2, S=2048, E=1024, H=16, D=64) on 8 Trainium2 NeuronCores.

Sharding: core c -> batch b=c//4, head-group g=c%4 (4 heads per core). Each core
runs the full per-group attention pipeline; the host sums the 4 partial
out-projections per batch element.

v2 kernel (bf16 matmuls, software-pipelined phases):
  - All matmul operands bf16 (weights + x converted on host); PSUM accumulation f32.
  - Projections produce qT/kT per head-pair ([128,512] tiles per q-block) and
    v per seq-chunk ([128,4,65] tiles, 65th col = exact ones for the softmax denom).
  - Biases are folded into the PSUM->SBUF epilogues (DVE tensor_scalar_add with a
    per-partition bias column for qT/kT; tensor_add with broadcast tiles for v/y),
    costing zero PE cycles while staying exact.
  - Attention per (pair, q-block): sT = kT_j @ qT_j per 128-key tile -> exp on ACT
    (bf16 out) -> po[65,512] accumulated with v stationary; row 64 = denominator.
  - Normalization: DVE reciprocal -> PE rank-1 broadcast (ones ⊗ recip) -> DVE mul
    writes aT slices (no DRAM round-trip).
  - Issue order software-pipelines the phases: pair-0 projections + v first, the
    pair-1 projections interleave into pair-0's attention loop, and each q-block's
    out-projection interleaves behind its attention, so PE stays busy during the
    ACT-heavy stretches. A tiny exp at t=0 preloads the ACT table during DMA waits.
"""

import numpy as np
import concourse.bass as bass
import concourse.tile as tile
from concourse import bacc, mybir
from concourse.bass_utils import run_bass_kernel_spmd

F32 = mybir.dt.float32
F32R = mybir.dt.float32r
BF16 = mybir.dt.bfloat16
NPBF16 = mybir.dt.np(mybir.dt.bfloat16)

S = 2048
E = 1024
HL = 4        # heads per core
D = 64
QB = 512
NQB = S // QB      # 4
NKT = S // 128     # 16
NKC = E // 128     # 8

_CACHE = {}


def _build(repeat=1):
    nc = bacc.Bacc("TRN2", target_bir_lowering=False, debug=False, num_devices=8)

    xT_d = nc.dram_tensor("xT", [E, S], BF16, kind="ExternalInput").ap()
    wqk_d = nc.dram_tensor("wqk", [E, 512], BF16, kind="ExternalInput").ap()
    bqk_d = nc.dram_tensor("bqk", [128, 4], F32, kind="ExternalInput").ap()
    wv_d = nc.dram_tensor("wv", [E, 256], BF16, kind="ExternalInput").ap()
    bv_d = nc.dram_tensor("bv", [1, 256], F32, kind="ExternalInput").ap()
    wo_d = nc.dram_tensor("wo", [256, E], BF16, kind="ExternalInput").ap()
    bo_d = nc.dram_tensor("bo", [1, E], F32, kind="ExternalInput").ap()
    onesc_d = nc.dram_tensor("onesc", [1, 64], F32R, kind="ExternalInput").ap()
    y_d = nc.dram_tensor("y", [S, E], F32, kind="ExternalOutput").ap()

    with tile.TileContext(nc) as tc:
        with (
            nc.allow_low_precision(reason="bf16 activations by design; f32 accum in PSUM"),
            tc.tile_pool(name="wpool", bufs=1) as wpool,
            tc.tile_pool(name="big", bufs=1) as big,
            tc.tile_pool(name="espool", bufs=5) as espool,
            tc.tile_pool(name="denpool", bufs=4) as denpool,
            tc.tile_pool(name="araw", bufs=4) as araw,
            tc.tile_pool(name="ypool", bufs=3) as ypool,
            tc.tile_pool(name="ps", bufs=2, space="PSUM") as ps,
            tc.tile_pool(name="pq", bufs=2, space="PSUM") as pq,
            tc.tile_pool(name="pr", bufs=2, space="PSUM") as pr,
        ):
            # ---- persistent weights (outside the repeat loop) ----
            # order: everything stage A needs first (wqk, bqk, wv, bv), then the rest
            wqk_t = []
            for k in range(NKC):
                w = wpool.tile([128, 512], BF16, name=f"wqk{k}")
                nc.sync.dma_start(w[:], wqk_d[k * 128:(k + 1) * 128, :])
                wqk_t.append(w)
            bqk_t = wpool.tile([128, 4], F32, name="bqk_t")
            nc.sync.dma_start(bqk_t[:], bqk_d[:])
            wv_t = []
            for k in range(NKC):
                w = wpool.tile([128, 256], BF16, name=f"wv{k}")
                nc.sync.dma_start(w[:], wv_d[k * 128:(k + 1) * 128, :])
                wv_t.append(w)
            bv_bc = wpool.tile([128, 256], F32, name="bv_bc")
            nc.sync.dma_start(bv_bc[:], bv_d[0:1, :].to_broadcast((128, 256)))
            onesc_t = wpool.tile([1, 64], F32R, name="onesc_t")
            nc.sync.dma_start(onesc_t[:], onesc_d[:])
            wo_t = []
            for c in range(2):
                w = wpool.tile([128, E], BF16, name=f"wo{c}")
                nc.sync.dma_start(w[:], wo_d[c * 128:(c + 1) * 128, :])
                wo_t.append(w)
            bo_bc = wpool.tile([128, E], F32, name="bo_bc")
            nc.sync.dma_start(bo_bc[:], bo_d[0:1, :].to_broadcast((128, E)))

            # ACT exp-table warmup: runs during the weight DMAs
            warm = wpool.tile([1, 8], F32, name="warm")
            nc.vector.memset(warm[:], 0.0)
            nc.scalar.activation(warm[:], warm[:], mybir.ActivationFunctionType.Exp)

            # ---- persistent per-iteration tiles ----
            # x_t[k][qb]: xT chunk [128, 512]
            x_t = [[big.tile([128, QB], BF16, name=f"x{k}_{qb}") for qb in range(NQB)]
                   for k in range(NKC)]
            # qk_t[m][qb]: m=0 qT pair0, 1 qT pair1, 2 kT pair0, 3 kT pair1
            qk_t = [[big.tile([128, QB], BF16, name=f"qk{m}_{qb}") for qb in range(NQB)]
                    for m in range(4)]
            vt = [big.tile([128, HL, 65], BF16, name=f"vt{st}") for st in range(NKT)]
            a_t = [big.tile([128, S], BF16, name=f"a{c}") for c in range(2)]

            def load_x():
                for qb in range(NQB):
                    for k in range(NKC):
                        nc.gpsimd.dma_start(
                            x_t[k][qb][:],
                            xT_d[k * 128:(k + 1) * 128, qb * QB:(qb + 1) * QB])

            def qk_proj(m, qb):
                p = pr.tile([128, QB], F32, name="prt", tag="r")
                for k in range(NKC):
                    nc.tensor.matmul(p[:], wqk_t[k][:, m * 128:(m + 1) * 128],
                                     x_t[k][qb][:], start=(k == 0), stop=(k == NKC - 1))
                nc.vector.tensor_scalar_add(qk_t[m][qb][:], p[:], bqk_t[:, m:m + 1])

            def v_proj(st):
                qb, sl = st // 4, st % 4
                p = pr.tile([128, QB], F32, name="prt", tag="r")
                pv = p[:, 0:256]
                for k in range(NKC):
                    nc.tensor.matmul(pv, x_t[k][qb][:, sl * 128:(sl + 1) * 128],
                                     wv_t[k][:], start=(k == 0), stop=(k == NKC - 1))
                nc.vector.tensor_add(
                    vt[st][:, :, 0:64],
                    pv.rearrange("p (h d) -> p h d", h=HL),
                    bv_bc[:].rearrange("p (h d) -> p h d", h=HL))
                nc.vector.memset(vt[st][:, :, 64:65], 1.0)

            def attn_block(pr_i, qb, interleave):
                """Scores+exp+po for head pair pr_i on q-block qb; `interleave` is a
                list of (after_kt, fn) callbacks issued inside the kt loop."""
                j0, j1 = 2 * pr_i, 2 * pr_i + 1
                po_a = pq.tile([65, QB], F32, name="po_a", tag="q")
                po_b = pq.tile([65, QB], F32, name="po_b", tag="q")
                cbs = dict(interleave)
                for kt in range(NKT):
                    qt_c, cc = kt // 4, (kt % 4) * 128
                    ps_t = ps.tile([128, 2 * QB], F32, name="ps_t", tag="s")
                    nc.tensor.matmul(ps_t[:, 0:QB], qk_t[2 + pr_i][qt_c][0:64, cc:cc + 128],
                                     qk_t[pr_i][qb][0:64, :], start=True, stop=True)
                    nc.tensor.matmul(ps_t[:, QB:2 * QB],
                                     qk_t[2 + pr_i][qt_c][64:128, cc:cc + 128],
                                     qk_t[pr_i][qb][64:128, :], start=True, stop=True)
                    es = espool.tile([128, 2 * QB], BF16, name="es")
                    nc.scalar.activation(es[:], ps_t[:], mybir.ActivationFunctionType.Exp)
                    nc.tensor.matmul(po_a[:], vt[kt][:, j0, :], es[:, 0:QB],
                                     start=(kt == 0), stop=(kt == NKT - 1))
                    nc.tensor.matmul(po_b[:], vt[kt][:, j1, :], es[:, QB:2 * QB],
                                     start=(kt == 0), stop=(kt == NKT - 1))
                    if kt in cbs:
                        cbs[kt]()
                for half, po_t in ((0, po_a), (1, po_b)):
                    den_r = denpool.tile([1, QB], F32R, name="den_r")
                    nc.vector.reciprocal(den_r[:], po_t[64:65, :])
                    a_raw = araw.tile([64, QB], BF16, name="a_raw")
                    nc.vector.tensor_copy(a_raw[:], po_t[0:64, :])
                    den_bc = pr.tile([64, QB], F32, name="den_bc", tag="r")
                    nc.tensor.matmul(den_bc[:], onesc_t[:], den_r[:], start=True, stop=True)
                    a_out = a_t[pr_i][half * 64:half * 64 + 64, qb * QB:(qb + 1) * QB]
                    nc.vector.tensor_mul(a_out, a_raw[:], den_bc[:])

            def out_proj_sl(st):
                for n in range(2):
                    p = pr.tile([128, QB], F32, name="prt", tag="r")
                    nc.tensor.matmul(p[:], a_t[0][:, st * 128:(st + 1) * 128],
                                     wo_t[0][:, n * QB:(n + 1) * QB],
                                     start=True, stop=False)
                    nc.tensor.matmul(p[:], a_t[1][:, st * 128:(st + 1) * 128],
                                     wo_t[1][:, n * QB:(n + 1) * QB],
                                     start=False, stop=True)
                    yt = ypool.tile([128, QB], F32, name="yt")
                    nc.vector.tensor_add(yt[:], p[:], bo_bc[:, n * QB:(n + 1) * QB])
                    nc.gpsimd.dma_start(
                        y_d[st * 128:(st + 1) * 128, n * QB:(n + 1) * QB], yt[:])

            def _iter_body():
                load_x()
                # stage A: pair-0 q/k projections + all of v
                for qb in range(NQB):
                    qk_proj(0, qb)
                    qk_proj(2, qb)
                    for sl in range(4):
                        v_proj(qb * 4 + sl)
                for qb in range(NQB):
                    qk_proj(1, qb)
                    qk_proj(3, qb)
                for qb in range(NQB):
                    attn_block(0, qb, [])
                for qb in range(NQB):
                    attn_block(1, qb, [])
                for st in range(16):
                    out_proj_sl(st)

            if repeat == 1:
                _iter_body()
            else:
                with tc.For_i(0, repeat):
                    _iter_body()

    nc.compile()
    return nc


def _shard_inputs(query, W_qkv, b_qkv, W_out, b_out):
    scale = np.float32(1.0 / np.sqrt(D))
    query = np.asarray(query, dtype=np.float32)
    W_qkv = np.asarray(W_qkv, dtype=np.float32)
    b_qkv = np.asarray(b_qkv, dtype=np.float32)
    W_out = np.asarray(W_out, dtype=np.float32)
    b_out = np.asarray(b_out, dtype=np.float32)

    W_q, W_k, W_v = W_qkv[:, :E], W_qkv[:, E:2 * E], W_qkv[:, 2 * E:]
    b_q, b_k, b_v = b_qkv[:E], b_qkv[E:2 * E], b_qkv[2 * E:]

    in_maps = []
    for c in range(8):
        b = c // 4
        g = c % 4
        hsl = slice(4 * g * D, (4 * g + 4) * D)
        wqk = np.empty((E, 512), np.float32)
        wqk[:, :256] = W_q[:, hsl] * scale
        wqk[:, 256:] = W_k[:, hsl]
        bqk_cols = np.empty((512,), np.float32)
        bqk_cols[:256] = b_q[hsl] * scale
        bqk_cols[256:] = b_k[hsl]
        bqk = np.ascontiguousarray(bqk_cols.reshape(4, 128).T)
        in_maps.append({
            "xT": np.ascontiguousarray(query[b].T).astype(NPBF16),
            "wqk": wqk.astype(NPBF16),
            "bqk": bqk,
            "wv": np.ascontiguousarray(W_v[:, hsl]).astype(NPBF16),
            "bv": np.ascontiguousarray(b_v[hsl]).reshape(1, 256),
            "wo": np.ascontiguousarray(W_out[hsl, :]).astype(NPBF16),
            "bo": (b_out if g == 0 else np.zeros_like(b_out)).reshape(1, E),
            "onesc": np.ones((1, 64), np.float32),
        })
    return in_maps


def kernel(query, W_qkv, b_qkv, W_out, b_out):
    if "nc" not in _CACHE:
        _CACHE["nc"] = _build()
    nc = _CACHE["nc"]
    in_maps = _shard_inputs(query, W_qkv, b_qkv, W_out, b_out)
    res = run_bass_kernel_spmd(nc, in_maps, list(range(8)))
    out = np.zeros((2, S, E), np.float32)
    for c in range(8):
        out[c // 4] += res.results[c]["y"]
    return out
